# revision 2
# baseline (speedup 1.0000x reference)
"""AttnBlock on 8 TRN2 NeuronCores -- F(4,3) x-Winograd variant.

Same structure as kernel_wino (F(2,3)): host-side input/weight transforms,
fp8 DoubleRow GEMMs with fp32 PSUM, v produced pre-transposed by swapping
conv operands, deferred softmax normalization. Differences:

  - x-dim tiles are 4 outputs wide (6-tap input windows, stride 4): 6 GEMM
    positions x 3 ky taps = 18 K-512 passes per 4 output columns vs the
    direct conv's 36 -- 2x fewer conv MACs, 1.33x fewer than F(2,3).
  - q/k GEMMs run N=256 (one sample's (y,tx) tiles); the two samples
    interleave inside each oc so every LDWEIGHTS serves two matmuls
    (N=256 alone would make the stream LDW-bound).
  - PSUM: six M planes of [P,256] f32 pack into three 1-bank [P,2,256]
    tiles per group; eight 1-bank tiles rotate, so two groups ping-pong.
  - the x output transform A^T has rows {1,1,1,1,1,0; 0,1,-1,2,-2,0;
    0,1,1,4,4,0; 0,1,-1,8,-8,1}: with M1/M3 staged to SBUF by ScalarE, DVE
    forms sp/sm = M1+-M2 and p34/m34 = M3+-M4 (one PSUM operand per op) and
    the four outputs are two adds + scalar_tensor_tensor fused multiply-adds.
  - weights scale x128 (the 1/6..1/24 G-transform coefficients would land
    subnormal at x32); the exp scale and the vT cast absorb it.
  - bias rides on M1 (A^T column 1 is all-ones), as before.

The pixel parity interleave is now 4-way: pixel m = 512*tc + 4p + jj; the
scores lhsT picks k columns via a (m four)-rearrange, and exps/vT pair
(jj0,jj1) / (jj2,jj3) per t-chunk for the DoubleRow contraction.
"""

import numpy as np
import ml_dtypes

import concourse.bass as bass
import concourse.tile as tile
from concourse import bacc, mybir
from concourse.bass_utils import run_bass_kernel_spmd

P = 128
B, C, H, W = 16, 512, 32, 32
NCORES = 8
S = B // NCORES
HP = WP = H + 2
NPIX = H * W
CC = C // P
OCH = (3 * C) // P
QKCH = 2 * CC
NT = 2
NTILE = 512
TX = W // 4          # 8 x-tiles
NP6 = 6              # winograd positions
KY = 3
NQK = H * TX         # 256: per-sample N for q/k GEMMs
VTC = 2              # v-conv t-chunks per sample (256 t / 128)
SU = 32.0            # weight scale into fp8 (q/k tails must stay under the
                     # e4m3 inf threshold; the G-coeff subnormals don't matter
                     # at this error budget)

BF16 = mybir.dt.bfloat16
F32 = mybir.dt.float32
F8 = mybir.dt.float8e4
EXP = mybir.ActivationFunctionType.Exp
DR = mybir.MatmulPerfMode.DoubleRow
MULT = mybir.AluOpType.mult
ADD = mybir.AluOpType.add

TRACE = False
LAST_EXEC_NS = None

_CACHED = {}


def build_nc(with_bias=True):
    nc = bacc.Bacc()
    vw_d = nc.declare_dram_parameter("vw", [NP6, 2, P, 2 * S * HP * TX], F8,
                                     isOutput=False)
    uw_d = nc.declare_dram_parameter("uw", [QKCH, P, NP6 * KY * 2 * 2 * P], F8,
                                     isOutput=False)
    uv_d = nc.declare_dram_parameter("uv", [P, NP6 * KY * 2 * 2 * CC * P], F8,
                                     isOutput=False)
    bqkv_d = nc.declare_dram_parameter("bqkv", [QKCH, P], BF16, isOutput=False)
    bv_d = nc.declare_dram_parameter("bv", [1, C], BF16, isOutput=False)
    wproj_d = nc.declare_dram_parameter("wproj", [2, P, 2, C], F8, isOutput=False)
    out_d = nc.declare_dram_parameter("out", [S, P, CC, NPIX], BF16, isOutput=True)

    with tile.TileContext(nc) as tc:
        with (
            tc.tile_pool(name="const", bufs=1) as constp,
            tc.tile_pool(name="resid", bufs=1) as resid,
            tc.tile_pool(name="stage", bufs=2) as stage,
        ):
            ones8 = constp.tile([P, 2, 16], F8, name="ones8")
            nc.vector.memset(ones8, 1.0)
            ones_nt = constp.tile([1, NTILE], BF16, name="ones_nt")
            nc.vector.memset(ones_nt, 1.0)
            ones_colT = constp.tile([1, P], BF16, name="ones_colT")
            nc.vector.memset(ones_colT, 1.0 / float(1 << 21))
            ones_col1 = constp.tile([1, P], BF16, name="ones_col1")
            nc.vector.memset(ones_col1, 1.0)

            v_in = {}
            v_vw = {}
            v_fl = {}
            for v in range(NP6):
                for j in range(2):
                    t = resid.tile([P, 2 * S * HP * TX], F8, tag="vin",
                                   bufs=NP6 * 2, name=f"vin_{v}_{j}")
                    v_in[(v, j)] = t
                    v_vw[(v, j)] = t.rearrange("p (i s y t) -> p i s y t",
                                               i=2, s=S, y=HP, t=TX)
                    v_fl[(v, j)] = t.rearrange("p (i r) -> p i r", i=2)

            bias_sb = constp.tile([1, QKCH, P], BF16, name="bias_sb")
            bv_sb = constp.tile([1, C], BF16, name="bv_sb")
            if with_bias:
                nc.sync.dma_start(bias_sb, bqkv_d[:])
                nc.sync.dma_start(bv_sb, bv_d[:])
            nc.sync.dma_start(v_in[(0, 0)], vw_d[0, 0])
            nc.sync.dma_start(v_in[(0, 1)], vw_d[0, 1])

            upool = tc.alloc_tile_pool(name="uw", bufs=1)
            u_sb = {}
            u_vw = {}
            for oc in range(QKCH):
                t = upool.tile([P, NP6 * KY * 2 * 2 * P], F8, tag="u",
                               bufs=4, name=f"u_{oc}")
                u_sb[oc] = t
                u_vw[oc] = t.rearrange("p (slot i co) -> p slot i co",
                                       slot=NP6 * KY * 2, i=2, co=P)
            nc.gpsimd.dma_start(u_sb[0], uw_d[0])
            for v in range(1, NP6):
                for j in range(2):
                    nc.gpsimd.dma_start(v_in[(v, j)], vw_d[v, j])
            nc.gpsimd.dma_start(u_sb[1], uw_d[1])

            uvpool = tc.alloc_tile_pool(name="uv", bufs=1)
            uv_sb = uvpool.tile([P, NP6 * KY * 2 * 2 * CC * P], F8, tag="uv",
                                bufs=1, name="uv")
            uv_vw = uv_sb.rearrange("p (slot i co) -> p slot i co",
                                    slot=NP6 * KY * 2, i=2, co=C)
            nc.gpsimd.dma_start(uv_sb, uv_d[:])
            wproj_sb = []
            for cj in range(2):
                t = resid.tile([P, 2, C], F8, tag="wproj", bufs=2, name=f"wproj_{cj}")
                nc.gpsimd.dma_start(t, wproj_d[cj])
                wproj_sb.append(t)
            for oc in range(2, QKCH):
                nc.gpsimd.dma_start(u_sb[oc], uw_d[oc])

            qk8 = {}
            for s in range(S):
                for w8 in ("q", "k"):
                    for j in range(2):
                        qk8[(s, w8, j)] = resid.tile(
                            [P, 2, NPIX], F8, tag="qk8", bufs=S * 4,
                            name=f"{w8}8_{s}_{j}")
            # vT pair tiles: key (s, tc, jp); [t-part, jj%2, co],
            # pixel m = 512*tc + 4p + (2*jp + jj%2)
            vT8_sb = {}
            for s in range(S):
                for tcn in range(VTC):
                    for jp in range(2):
                        vT8_sb[(s, tcn, jp)] = resid.tile(
                            [P, 2, C], F8, tag="vt", bufs=S * VTC * 2,
                            name=f"vt8_{s}_{tcn}_{jp}")

            def drain(mm01, mm23, mm45, n_, emit):
                # outs from A^T rows; ScalarE stages M1/M3, DVE chains with
                # one PSUM operand per op; emit(j, src_sbuf_f32_or_bf16...)
                c1 = stage.tile([P, n_], F32, tag="c1", name=f"c1_{drain.n}")
                c3 = stage.tile([P, n_], F32, tag="c3", name=f"c3_{drain.n}")
                sp = stage.tile([P, n_], F32, tag="sp", name=f"sp_{drain.n}")
                sm = stage.tile([P, n_], F32, tag="sm", name=f"sm_{drain.n}")
                p34 = stage.tile([P, n_], F32, tag="p34", name=f"p34_{drain.n}")
                m34 = stage.tile([P, n_], F32, tag="m34", name=f"m34_{drain.n}")
                t0 = stage.tile([P, n_], F32, tag="t0", name=f"t0_{drain.n}")
                t3 = stage.tile([P, n_], F32, tag="t3", name=f"t3_{drain.n}")
                o = [stage.tile([P, n_], BF16, tag=f"o{j}",
                                name=f"o{j}_{drain.n}") for j in range(4)]
                drain.n += 1
                nc.vector.tensor_copy(out=c1, in_=mm01[:, 1])
                nc.vector.tensor_copy(out=c3, in_=mm23[:, 1])
                nc.vector.tensor_add(sp, c1, mm23[:, 0])
                nc.vector.tensor_sub(sm, c1, mm23[:, 0])
                nc.vector.tensor_add(p34, c3, mm45[:, 0])
                nc.vector.tensor_sub(m34, c3, mm45[:, 0])
                nc.vector.tensor_add(t0, mm01[:, 0], sp)
                nc.vector.tensor_add(o[0], t0, p34)
                nc.vector.scalar_tensor_tensor(o[1], m34, 2.0, sm,
                                               op0=MULT, op1=ADD)
                nc.vector.scalar_tensor_tensor(o[2], p34, 4.0, sp,
                                               op0=MULT, op1=ADD)
                nc.vector.scalar_tensor_tensor(t3, m34, 8.0, mm45[:, 1],
                                               op0=MULT, op1=ADD)
                nc.vector.tensor_add(o[3], t3, sm)
                for j in range(4):
                    emit(j, o[j])
            drain.n = 0

            mpool = tc.alloc_tile_pool(name="mconv", bufs=1, space="PSUM")

            # q/k conv: M = [co, (y,tx)], N=256/sample, samples share LDW
            for oc in range(QKCH):
                mts = {}
                for s in range(S):
                    mts[s] = [mpool.tile([P, 2, NQK], F32, tag="m", bufs=8,
                                         name=f"m_{oc}_{s}_{g}")
                              for g in range(3)]
                    if with_bias:
                        nc.tensor.matmul(mts[s][0][:, 1], lhsT=bias_sb[0:1, oc],
                                         rhs=ones_nt[:, 0:NQK],
                                         start=True, stop=False)
                for v in range(NP6):
                    for ky in range(KY):
                        for j in range(2):
                            for s in range(S):
                                pl = mts[s][v // 2][:, v % 2]
                                nc.tensor.matmul(
                                    pl,
                                    lhsT=u_vw[oc][:, (v * KY + ky) * 2 + j],
                                    rhs=v_vw[(v, j)][:, :, s, ky:ky + H, :],
                                    start=((v != 1 or not with_bias)
                                           and ky == 0 and j == 0),
                                    stop=(ky == KY - 1 and j == 1),
                                    perf_mode=DR)
                for s in range(S):
                    if oc < CC:
                        dv, ii = qk8[(s, "q", oc // 2)], oc % 2
                    else:
                        dv, ii = qk8[(s, "k", (oc - CC) // 2)], (oc - CC) % 2
                    dvv = dv.rearrange("p a (y tx four) -> p a y tx four",
                                       four=4, tx=TX)

                    def emit_qk(j, o, dvv=dvv, ii=ii):
                        nc.scalar.copy(
                            dvv[:, ii, :, :, j],
                            o.rearrange("p (y tx) -> p y tx", tx=TX))
                    drain(mts[s][0], mts[s][1], mts[s][2], NQK, emit_qk)

            # v conv: operands swapped -> M = [t(128), co]; co in halves of
            # 256 sharing each LDW; vT8 = out/SU via the ScalarE cast
            for s in range(S):
                for tcn in range(VTC):
                    mts = {}
                    for ch in range(2):
                        mts[ch] = [mpool.tile([P, 2, 256], F32, tag="m", bufs=8,
                                              name=f"mv_{s}_{tcn}_{ch}_{g}")
                                   for g in range(3)]
                        if with_bias:
                            nc.tensor.matmul(
                                mts[ch][0][:, 1], lhsT=ones_col1,
                                rhs=bv_sb[:, ch * 256:(ch + 1) * 256],
                                start=True, stop=False)
                    for v in range(NP6):
                        for ky in range(KY):
                            for j in range(2):
                                off = s * (HP * TX) + (16 * tcn + ky) * TX
                                for ch in range(2):
                                    pl = mts[ch][v // 2][:, v % 2]
                                    nc.tensor.matmul(
                                        pl,
                                        lhsT=v_fl[(v, j)][:, :, off:off + P],
                                        rhs=uv_vw[:, (v * KY + ky) * 2 + j,
                                                  :, ch * 256:(ch + 1) * 256],
                                        start=((v != 1 or not with_bias)
                                               and ky == 0 and j == 0),
                                        stop=(ky == KY - 1 and j == 1),
                                        perf_mode=DR)
                    for ch in range(2):
                        def emit_v(j, o, s=s, tcn=tcn, ch=ch):
                            nc.scalar.mul(
                                vT8_sb[(s, tcn, j // 2)][:, j % 2,
                                                         ch * 256:(ch + 1) * 256],
                                o, 1.0 / SU)
                        drain(mts[ch][0], mts[ch][1], mts[ch][2], 256, emit_v)

            mpool.release()
            uvpool.release()
            upool.release()

            # ---- attention ----
            with (
                tc.tile_pool(name="attn", bufs=1) as attn,
                tc.tile_pool(name="stream", bufs=2) as stream,
                tc.tile_pool(name="psm", bufs=6, space="PSUM") as psm,
                tc.tile_pool(name="pss", bufs=2, space="PSUM") as pss,
            ):
                exps8 = {}
                for s in range(S):
                    for tcn in range(VTC):
                        for jp in range(2):
                            exps8[(s, tcn, jp)] = attn.tile(
                                [P, 2, NPIX], F8, tag="exps",
                                bufs=S * VTC * 2, name=f"exps_{s}_{tcn}_{jp}")
                for s in range(S):
                    for tcn in range(VTC):
                        for jp in range(2):
                            for j2 in range(2):
                                jj = jp * 2 + j2
                                ps_nt = [psm.tile([P, NTILE], F32, tag="mm",
                                                  name=f"ps_sc_{s}_{tcn}_{jj}_{nt}")
                                         for nt in range(NT)]
                                for j in range(2):
                                    kv = qk8[(s, "k", j)].rearrange(
                                        "p a (m four) -> p a m four", four=4)
                                    lhsT = kv[:, :, P * tcn:P * (tcn + 1), jj]
                                    for nt in range(NT):
                                        nc.tensor.matmul(
                                            ps_nt[nt], lhsT=lhsT,
                                            rhs=qk8[(s, "q", j)][
                                                :, :, nt * NTILE:(nt + 1) * NTILE],
                                            start=(j == 0), stop=(j == 1),
                                            perf_mode=DR)
                                for nt in range(NT):
                                    nc.scalar.activation(
                                        exps8[(s, tcn, jp)][
                                            :, j2, nt * NTILE:(nt + 1) * NTILE],
                                        ps_nt[nt], EXP,
                                        scale=float(C) ** -0.5 / (SU * SU))

                r_bfs = {}
                for s in range(S):
                    r_sb = stream.tile([1, NPIX], F32, tag="r", bufs=2,
                                       name=f"r_{s}")
                    r_bf = stream.tile([1, NPIX], BF16, tag="rb16", bufs=2,
                                       name=f"rb16_{s}")
                    ps_sums = [pss.tile([1, NTILE], F32, tag="sum",
                                        name=f"ps_sum_{s}_{nt}") for nt in range(NT)]
                    keys = [(tcn, jp) for tcn in range(VTC) for jp in range(2)]
                    for ki, (tcn, jp) in enumerate(keys):
                        for nt in range(NT):
                            nc.tensor.matmul(
                                ps_sums[nt], lhsT=ones8[:, :, 0:1],
                                rhs=exps8[(s, tcn, jp)][:, :,
                                                        nt * NTILE:(nt + 1) * NTILE],
                                start=(ki == 0), stop=(ki == len(keys) - 1),
                                perf_mode=DR)
                    for nt in range(NT):
                        nc.vector.reciprocal_approx_fast(
                            out=r_sb[:, nt * NTILE:(nt + 1) * NTILE],
                            in_=ps_sums[nt])
                    nc.scalar.copy(out=r_bf, in_=r_sb)
                    r_bfs[s] = r_bf

                rbc = {}
                for s in range(S):
                    for nt in range(NT):
                        ps_b = psm.tile([P, NTILE], F32, tag="mm",
                                        name=f"ps_rb_{s}_{nt}")
                        nc.tensor.matmul(ps_b, lhsT=ones_colT,
                                         rhs=r_bfs[s][:, nt * NTILE:(nt + 1) * NTILE],
                                         start=True, stop=True)
                        rb = stream.tile([P, NTILE], F32, tag="rbc", bufs=4,
                                         name=f"rbc_{s}_{nt}")
                        nc.scalar.copy(out=rb, in_=ps_b)
                        rbc[(s, nt)] = rb

                hN = {}
                for s in range(S):
                    for cj in range(2):
                        hN[(s, cj)] = attn.tile([P, 2, NPIX], F8, tag="hn",
                                                bufs=2 * S, name=f"hn_{s}_{cj}")
                for s in range(S):
                    for cc in range(CC):
                        ps_h = [psm.tile([P, NTILE], F32, tag="mm",
                                         name=f"ps_h_{s}_{cc}_{nt}")
                                for nt in range(NT)]
                        keys = [(tcn, jp) for tcn in range(VTC) for jp in range(2)]
                        for ki, (tcn, jp) in enumerate(keys):
                            lhsT = vT8_sb[(s, tcn, jp)][:, :, cc * P:(cc + 1) * P]
                            for nt in range(NT):
                                nc.tensor.matmul(
                                    ps_h[nt], lhsT=lhsT,
                                    rhs=exps8[(s, tcn, jp)][
                                        :, :, nt * NTILE:(nt + 1) * NTILE],
                                    start=(ki == 0), stop=(ki == len(keys) - 1),
                                    perf_mode=DR)
                        for nt in range(NT):
                            nc.vector.tensor_copy(
                                out=hN[(s, cc // 2)][:, cc % 2,
                                                     nt * NTILE:(nt + 1) * NTILE],
                                in_=ps_h[nt])

                for s in range(S):
                    o_t = stream.tile([P, CC, NPIX], BF16, tag="ostage", bufs=2,
                                      name=f"o_{s}")
                    for oc in range(CC):
                        ps_p = [psm.tile([P, NTILE], F32, tag="mm",
                                         name=f"ps_p_{s}_{oc}_{nt}")
                                for nt in range(NT)]
                        for cj in range(2):
                            lhsT = wproj_sb[cj][:, :, oc * P:(oc + 1) * P]
                            for nt in range(NT):
                                nc.tensor.matmul(
                                    ps_p[nt], lhsT=lhsT,
                                    rhs=hN[(s, cj)][:, :, nt * NTILE:(nt + 1) * NTILE],
                                    start=(cj == 0), stop=(cj == 1),
                                    perf_mode=DR)
                        for nt in range(NT):
                            sl = slice(nt * NTILE, (nt + 1) * NTILE)
                            nc.vector.tensor_mul(out=o_t[:, oc, sl], in0=ps_p[nt],
                                                 in1=rbc[(s, nt)])
                        nc.scalar.dma_start(out_d[s, :, oc], o_t[:, oc])

    nc.finalize()
    return nc


BT43 = np.array([
    [4, 0, -5, 0, 1, 0],
    [0, -4, -4, 1, 1, 0],
    [0, 4, -4, -1, 1, 0],
    [0, -2, -1, 2, 1, 0],
    [0, 2, -1, -2, 1, 0],
    [0, 4, 0, -5, 0, 1]], np.float32)
G43 = np.array([
    [1 / 4, 0, 0],
    [-1 / 6, -1 / 6, -1 / 6],
    [-1 / 6, 1 / 6, -1 / 6],
    [1 / 24, 1 / 12, 1 / 6],
    [1 / 24, -1 / 12, 1 / 6],
    [0, 0, 1]], np.float32)


def prep_inputs(x, w_qkv, b_qkv):
    e4 = ml_dtypes.float8_e4m3
    xpad = np.zeros((B, C, HP, WP), np.float32)
    xpad[:, :, 1:H + 1, 1:W + 1] = x

    taps = np.stack([xpad[:, :, :, a:a + 4 * TX:4][:, :, :, :TX]
                     for a in range(6)])          # [6, B, C, HP, TX]
    V = np.tensordot(BT43, taps, axes=([1], [0]))  # [6, B, C, HP, TX]
    vw = np.ascontiguousarray(
        V.reshape(NP6, B, 2, 2, P, HP, TX)
        .transpose(0, 2, 4, 3, 1, 5, 6)).astype(e4)  # [6, 2, P, 2, B, HP, TX]

    u6 = np.tensordot(w_qkv * SU, G43, axes=([3], [1]))  # [co, ci, ky, 6]
    uw = np.ascontiguousarray(
        u6[:2 * C].reshape(QKCH, P, 2, 2, P, KY, NP6)
        .transpose(0, 4, 6, 5, 2, 3, 1)
        .reshape(QKCH, P, NP6 * KY * 2 * 2 * P)).astype(e4)
    uv = np.ascontiguousarray(
        u6[2 * C:].reshape(C, 2, 2, P, KY, NP6)
        .transpose(3, 5, 4, 1, 2, 0)
        .reshape(P, NP6 * KY * 2 * 2 * C)).astype(e4)
    bqkv = np.ascontiguousarray((b_qkv[:2 * C] * SU).reshape(QKCH, P)).astype(
        ml_dtypes.bfloat16)
    bv = np.ascontiguousarray((b_qkv[2 * C:] * SU).reshape(1, C)).astype(
        ml_dtypes.bfloat16)
    return vw, uw, uv, bqkv, bv


def kernel(x, w_qkv, b_qkv, w_proj, b_proj, gn_gamma=None, gn_beta=None):
    global LAST_EXEC_NS
    x = np.asarray(x, np.float32)
    w_qkv = np.asarray(w_qkv, np.float32)
    b_qkv = np.asarray(b_qkv, np.float32)
    w_proj = np.asarray(w_proj, np.float32)
    b_proj = np.asarray(b_proj, np.float32)

    with_bias = bool(np.any(b_qkv))
    if with_bias not in _CACHED:
        _CACHED[with_bias] = build_nc(with_bias=with_bias)
    nc = _CACHED[with_bias]

    e4 = ml_dtypes.float8_e4m3
    vw, uw, uv, bqkv, bv = prep_inputs(x, w_qkv, b_qkv)
    wproj = np.ascontiguousarray(
        (w_proj[:, :, 0, 0].T * float(1 << 21))
        .reshape(2, 2, P, C).transpose(0, 2, 1, 3)).astype(e4)

    in_maps = []
    for core in range(NCORES):
        sl = slice(core * S, (core + 1) * S)
        in_maps.append({
            "vw": np.ascontiguousarray(vw[:, :, :, :, sl]).reshape(NP6, 2, P, -1),
            "uw": uw,
            "uv": uv,
            "wproj": wproj,
            "bqkv": bqkv,
            "bv": bv,
        })

    res = run_bass_kernel_spmd(nc, in_maps, list(range(NCORES)), trace=TRACE)
    LAST_EXEC_NS = res.exec_time_ns
    h = np.stack([np.asarray(res.results[c]["out"], np.float32)
                  for c in range(NCORES)])
    h = h.reshape(B, P, CC, NPIX).transpose(0, 2, 1, 3).reshape(B, C, H, W)
    out = x + h + b_proj[None, :, None, None]
    return np.ascontiguousarray(out).astype(np.float32, copy=False)


# revision 3
# speedup vs baseline: 1.0005x; 1.0005x over previous
"""AttnBlock on 8 TRN2 NeuronCores -- F(4,3) x-Winograd variant.

Same structure as kernel_wino (F(2,3)): host-side input/weight transforms,
fp8 DoubleRow GEMMs with fp32 PSUM, v produced pre-transposed by swapping
conv operands, deferred softmax normalization. Differences:

  - x-dim tiles are 4 outputs wide (6-tap input windows, stride 4): 6 GEMM
    positions x 3 ky taps = 18 K-512 passes per 4 output columns vs the
    direct conv's 36 -- 2x fewer conv MACs, 1.33x fewer than F(2,3).
  - q/k GEMMs run N=256 (one sample's (y,tx) tiles); the two samples
    interleave inside each oc so every LDWEIGHTS serves two matmuls
    (N=256 alone would make the stream LDW-bound).
  - PSUM: six M planes of [P,256] f32 pack into three 1-bank [P,2,256]
    tiles per group; eight 1-bank tiles rotate, so two groups ping-pong.
  - the x output transform A^T has rows {1,1,1,1,1,0; 0,1,-1,2,-2,0;
    0,1,1,4,4,0; 0,1,-1,8,-8,1}: with M1/M3 staged to SBUF by ScalarE, DVE
    forms sp/sm = M1+-M2 and p34/m34 = M3+-M4 (one PSUM operand per op) and
    the four outputs are two adds + scalar_tensor_tensor fused multiply-adds.
  - weights scale x128 (the 1/6..1/24 G-transform coefficients would land
    subnormal at x32); the exp scale and the vT cast absorb it.
  - bias rides on M1 (A^T column 1 is all-ones), as before.

The pixel parity interleave is now 4-way: pixel m = 512*tc + 4p + jj; the
scores lhsT picks k columns via a (m four)-rearrange, and exps/vT pair
(jj0,jj1) / (jj2,jj3) per t-chunk for the DoubleRow contraction.
"""

import numpy as np
import ml_dtypes

import concourse.bass as bass
import concourse.tile as tile
from concourse import bacc, mybir
from concourse.bass_utils import run_bass_kernel_spmd

P = 128
B, C, H, W = 16, 512, 32, 32
NCORES = 8
S = B // NCORES
HP = WP = H + 2
NPIX = H * W
CC = C // P
OCH = (3 * C) // P
QKCH = 2 * CC
NT = 2
NTILE = 512
TX = W // 4          # 8 x-tiles
NP6 = 6              # winograd positions
KY = 3
NQK = H * TX         # 256: per-sample N for q/k GEMMs
VTC = 2              # v-conv t-chunks per sample (256 t / 128)
SU = 32.0            # weight scale into fp8 (q/k tails must stay under the
                     # e4m3 inf threshold; the G-coeff subnormals don't matter
                     # at this error budget)

BF16 = mybir.dt.bfloat16
F32 = mybir.dt.float32
F8 = mybir.dt.float8e4
EXP = mybir.ActivationFunctionType.Exp
DR = mybir.MatmulPerfMode.DoubleRow
MULT = mybir.AluOpType.mult
ADD = mybir.AluOpType.add

TRACE = False
LAST_EXEC_NS = None

_CACHED = {}


def build_nc(with_bias=True):
    nc = bacc.Bacc()
    vw_d = nc.declare_dram_parameter("vw", [NP6, 2, P, 2 * S * HP * TX], F8,
                                     isOutput=False)
    uw_d = nc.declare_dram_parameter("uw", [QKCH, P, NP6 * KY * 2 * 2 * P], F8,
                                     isOutput=False)
    uv_d = nc.declare_dram_parameter("uv", [P, NP6 * KY * 2 * 2 * CC * P], F8,
                                     isOutput=False)
    bqkv_d = nc.declare_dram_parameter("bqkv", [QKCH, P], BF16, isOutput=False)
    bv_d = nc.declare_dram_parameter("bv", [1, C], BF16, isOutput=False)
    wproj_d = nc.declare_dram_parameter("wproj", [2, P, 2, C], F8, isOutput=False)
    out_d = nc.declare_dram_parameter("out", [S, P, CC, NPIX], BF16, isOutput=True)

    with tile.TileContext(nc) as tc:
        with (
            tc.tile_pool(name="const", bufs=1) as constp,
            tc.tile_pool(name="resid", bufs=1) as resid,
            tc.tile_pool(name="stage", bufs=2) as stage,
        ):
            ones8 = constp.tile([P, 2, 16], F8, name="ones8")
            nc.vector.memset(ones8, 1.0)
            ones_nt = constp.tile([1, NTILE], BF16, name="ones_nt")
            nc.vector.memset(ones_nt, 1.0)
            ones_colT = constp.tile([1, P], BF16, name="ones_colT")
            nc.vector.memset(ones_colT, 1.0 / float(1 << 21))
            ones_col1 = constp.tile([1, P], BF16, name="ones_col1")
            nc.vector.memset(ones_col1, 1.0)

            v_in = {}
            v_vw = {}
            v_fl = {}
            for v in range(NP6):
                for j in range(2):
                    t = resid.tile([P, 2 * S * HP * TX], F8, tag="vin",
                                   bufs=NP6 * 2, name=f"vin_{v}_{j}")
                    v_in[(v, j)] = t
                    v_vw[(v, j)] = t.rearrange("p (i s y t) -> p i s y t",
                                               i=2, s=S, y=HP, t=TX)
                    v_fl[(v, j)] = t.rearrange("p (i r) -> p i r", i=2)

            bias_sb = constp.tile([1, QKCH, P], BF16, name="bias_sb")
            bv_sb = constp.tile([1, C], BF16, name="bv_sb")
            if with_bias:
                nc.sync.dma_start(bias_sb, bqkv_d[:])
                nc.sync.dma_start(bv_sb, bv_d[:])
            nc.sync.dma_start(v_in[(0, 0)], vw_d[0, 0])
            nc.sync.dma_start(v_in[(0, 1)], vw_d[0, 1])

            upool = tc.alloc_tile_pool(name="uw", bufs=1)
            u_sb = {}
            u_vw = {}
            for oc in range(QKCH):
                t = upool.tile([P, NP6 * KY * 2 * 2 * P], F8, tag="u",
                               bufs=4, name=f"u_{oc}")
                u_sb[oc] = t
                u_vw[oc] = t.rearrange("p (slot i co) -> p slot i co",
                                       slot=NP6 * KY * 2, i=2, co=P)
            nc.gpsimd.dma_start(u_sb[0], uw_d[0])
            for v in range(1, NP6):
                for j in range(2):
                    nc.gpsimd.dma_start(v_in[(v, j)], vw_d[v, j])
            nc.gpsimd.dma_start(u_sb[1], uw_d[1])

            # uv (3.1MB) is not needed until the first v-group (~50us in);
            # keep it behind the next few U tiles so they don't stall oc 2-4
            uvpool = tc.alloc_tile_pool(name="uv", bufs=1)
            uv_sb = uvpool.tile([P, NP6 * KY * 2 * 2 * CC * P], F8, tag="uv",
                                bufs=1, name="uv")
            uv_vw = uv_sb.rearrange("p (slot i co) -> p slot i co",
                                    slot=NP6 * KY * 2, i=2, co=C)
            for oc in range(2, 5):
                nc.gpsimd.dma_start(u_sb[oc], uw_d[oc])
            nc.gpsimd.dma_start(uv_sb, uv_d[:])
            wproj_sb = []
            for cj in range(2):
                t = resid.tile([P, 2, C], F8, tag="wproj", bufs=2, name=f"wproj_{cj}")
                nc.gpsimd.dma_start(t, wproj_d[cj])
                wproj_sb.append(t)
            for oc in range(5, QKCH):
                nc.gpsimd.dma_start(u_sb[oc], uw_d[oc])

            qk8 = {}
            for s in range(S):
                for w8 in ("q", "k"):
                    for j in range(2):
                        qk8[(s, w8, j)] = resid.tile(
                            [P, 2, NPIX], F8, tag="qk8", bufs=S * 4,
                            name=f"{w8}8_{s}_{j}")
            # vT pair tiles: key (s, tc, jp); [t-part, jj%2, co],
            # pixel m = 512*tc + 4p + (2*jp + jj%2)
            vT8_sb = {}
            for s in range(S):
                for tcn in range(VTC):
                    for jp in range(2):
                        vT8_sb[(s, tcn, jp)] = resid.tile(
                            [P, 2, C], F8, tag="vt", bufs=S * VTC * 2,
                            name=f"vt8_{s}_{tcn}_{jp}")

            def drain(mm01, mm23, mm45, n_, emit):
                # outs from A^T rows; ScalarE stages M1/M3, DVE chains with
                # one PSUM operand per op; emit(j, src_sbuf_f32_or_bf16...)
                c1 = stage.tile([P, n_], F32, tag="c1", name=f"c1_{drain.n}")
                c3 = stage.tile([P, n_], F32, tag="c3", name=f"c3_{drain.n}")
                sp = stage.tile([P, n_], F32, tag="sp", name=f"sp_{drain.n}")
                sm = stage.tile([P, n_], F32, tag="sm", name=f"sm_{drain.n}")
                p34 = stage.tile([P, n_], F32, tag="p34", name=f"p34_{drain.n}")
                m34 = stage.tile([P, n_], F32, tag="m34", name=f"m34_{drain.n}")
                t0 = stage.tile([P, n_], F32, tag="t0", name=f"t0_{drain.n}")
                t3 = stage.tile([P, n_], F32, tag="t3", name=f"t3_{drain.n}")
                o = [stage.tile([P, n_], BF16, tag=f"o{j}",
                                name=f"o{j}_{drain.n}") for j in range(4)]
                drain.n += 1
                nc.vector.tensor_copy(out=c1, in_=mm01[:, 1])
                nc.vector.tensor_copy(out=c3, in_=mm23[:, 1])
                nc.vector.tensor_add(sp, c1, mm23[:, 0])
                nc.vector.tensor_sub(sm, c1, mm23[:, 0])
                nc.vector.tensor_add(p34, c3, mm45[:, 0])
                nc.vector.tensor_sub(m34, c3, mm45[:, 0])
                nc.vector.tensor_add(t0, mm01[:, 0], sp)
                nc.vector.tensor_add(o[0], t0, p34)
                nc.vector.scalar_tensor_tensor(o[1], m34, 2.0, sm,
                                               op0=MULT, op1=ADD)
                nc.vector.scalar_tensor_tensor(o[2], p34, 4.0, sp,
                                               op0=MULT, op1=ADD)
                nc.vector.scalar_tensor_tensor(t3, m34, 8.0, mm45[:, 1],
                                               op0=MULT, op1=ADD)
                nc.vector.tensor_add(o[3], t3, sm)
                for j in range(4):
                    emit(j, o[j])
            drain.n = 0

            mpool = tc.alloc_tile_pool(name="mconv", bufs=1, space="PSUM")

            # q/k conv: M = [co, (y,tx)], N=256/sample, samples share LDW
            def qk_group(oc):
                mts = {}
                for s in range(S):
                    mts[s] = [mpool.tile([P, 2, NQK], F32, tag="m", bufs=8,
                                         name=f"m_{oc}_{s}_{g}")
                              for g in range(3)]
                    if with_bias:
                        nc.tensor.matmul(mts[s][0][:, 1], lhsT=bias_sb[0:1, oc],
                                         rhs=ones_nt[:, 0:NQK],
                                         start=True, stop=False)
                for v in range(NP6):
                    for ky in range(KY):
                        for j in range(2):
                            for s in range(S):
                                pl = mts[s][v // 2][:, v % 2]
                                nc.tensor.matmul(
                                    pl,
                                    lhsT=u_vw[oc][:, (v * KY + ky) * 2 + j],
                                    rhs=v_vw[(v, j)][:, :, s, ky:ky + H, :],
                                    start=((v != 1 or not with_bias)
                                           and ky == 0 and j == 0),
                                    stop=(ky == KY - 1 and j == 1),
                                    perf_mode=DR)
                for s in range(S):
                    if oc < CC:
                        dv, ii = qk8[(s, "q", oc // 2)], oc % 2
                    else:
                        dv, ii = qk8[(s, "k", (oc - CC) // 2)], (oc - CC) % 2
                    dvv = dv.rearrange("p a (y tx four) -> p a y tx four",
                                       four=4, tx=TX)

                    def emit_qk(j, o, dvv=dvv, ii=ii):
                        nc.scalar.copy(
                            dvv[:, ii, :, :, j],
                            o.rearrange("p (y tx) -> p y tx", tx=TX))
                    drain(mts[s][0], mts[s][1], mts[s][2], NQK, emit_qk)

            # v conv: operands swapped -> M = [t(128), co]; co in halves of
            # 256 sharing each LDW; vT8 = out/SU via the ScalarE cast
            def v_group(s, tcn):
                if True:
                    mts = {}
                    for ch in range(2):
                        mts[ch] = [mpool.tile([P, 2, 256], F32, tag="m", bufs=8,
                                              name=f"mv_{s}_{tcn}_{ch}_{g}")
                                   for g in range(3)]
                        if with_bias:
                            nc.tensor.matmul(
                                mts[ch][0][:, 1], lhsT=ones_col1,
                                rhs=bv_sb[:, ch * 256:(ch + 1) * 256],
                                start=True, stop=False)
                    for v in range(NP6):
                        for ky in range(KY):
                            for j in range(2):
                                off = s * (HP * TX) + (16 * tcn + ky) * TX
                                for ch in range(2):
                                    pl = mts[ch][v // 2][:, v % 2]
                                    nc.tensor.matmul(
                                        pl,
                                        lhsT=v_fl[(v, j)][:, :, off:off + P],
                                        rhs=uv_vw[:, (v * KY + ky) * 2 + j,
                                                  :, ch * 256:(ch + 1) * 256],
                                        start=((v != 1 or not with_bias)
                                               and ky == 0 and j == 0),
                                        stop=(ky == KY - 1 and j == 1),
                                        perf_mode=DR)
                    for ch in range(2):
                        def emit_v(j, o, s=s, tcn=tcn, ch=ch):
                            nc.scalar.mul(
                                vT8_sb[(s, tcn, j // 2)][:, j % 2,
                                                         ch * 256:(ch + 1) * 256],
                                o, 1.0 / SU)
                        drain(mts[ch][0], mts[ch][1], mts[ch][2], 256, emit_v)

            # interleave the v-groups among the later q/k groups so their
            # drain/cast load doesn't pile up at the conv->attention boundary
            vkeys = [(s, tcn) for s in range(S) for tcn in range(VTC)]
            for oc in range(4):
                qk_group(oc)
            for i, oc in enumerate(range(4, QKCH)):
                qk_group(oc)
                v_group(*vkeys[i])

            mpool.release()
            uvpool.release()
            upool.release()

            # ---- attention ----
            with (
                tc.tile_pool(name="attn", bufs=1) as attn,
                tc.tile_pool(name="stream", bufs=2) as stream,
                tc.tile_pool(name="psm", bufs=6, space="PSUM") as psm,
                tc.tile_pool(name="pss", bufs=2, space="PSUM") as pss,
            ):
                exps8 = {}
                for s in range(S):
                    for tcn in range(VTC):
                        for jp in range(2):
                            exps8[(s, tcn, jp)] = attn.tile(
                                [P, 2, NPIX], F8, tag="exps",
                                bufs=S * VTC * 2, name=f"exps_{s}_{tcn}_{jp}")
                for s in range(S):
                    for tcn in range(VTC):
                        for jp in range(2):
                            for j2 in range(2):
                                jj = jp * 2 + j2
                                ps_nt = [psm.tile([P, NTILE], F32, tag="mm",
                                                  name=f"ps_sc_{s}_{tcn}_{jj}_{nt}")
                                         for nt in range(NT)]
                                for j in range(2):
                                    kv = qk8[(s, "k", j)].rearrange(
                                        "p a (m four) -> p a m four", four=4)
                                    lhsT = kv[:, :, P * tcn:P * (tcn + 1), jj]
                                    for nt in range(NT):
                                        nc.tensor.matmul(
                                            ps_nt[nt], lhsT=lhsT,
                                            rhs=qk8[(s, "q", j)][
                                                :, :, nt * NTILE:(nt + 1) * NTILE],
                                            start=(j == 0), stop=(j == 1),
                                            perf_mode=DR)
                                for nt in range(NT):
                                    nc.scalar.activation(
                                        exps8[(s, tcn, jp)][
                                            :, j2, nt * NTILE:(nt + 1) * NTILE],
                                        ps_nt[nt], EXP,
                                        scale=float(C) ** -0.5 / (SU * SU))

                r_bfs = {}
                for s in range(S):
                    r_sb = stream.tile([1, NPIX], F32, tag="r", bufs=2,
                                       name=f"r_{s}")
                    r_bf = stream.tile([1, NPIX], BF16, tag="rb16", bufs=2,
                                       name=f"rb16_{s}")
                    ps_sums = [pss.tile([1, NTILE], F32, tag="sum",
                                        name=f"ps_sum_{s}_{nt}") for nt in range(NT)]
                    keys = [(tcn, jp) for tcn in range(VTC) for jp in range(2)]
                    for ki, (tcn, jp) in enumerate(keys):
                        for nt in range(NT):
                            nc.tensor.matmul(
                                ps_sums[nt], lhsT=ones8[:, :, 0:1],
                                rhs=exps8[(s, tcn, jp)][:, :,
                                                        nt * NTILE:(nt + 1) * NTILE],
                                start=(ki == 0), stop=(ki == len(keys) - 1),
                                perf_mode=DR)
                    for nt in range(NT):
                        nc.vector.reciprocal_approx_fast(
                            out=r_sb[:, nt * NTILE:(nt + 1) * NTILE],
                            in_=ps_sums[nt])
                    nc.scalar.copy(out=r_bf, in_=r_sb)
                    r_bfs[s] = r_bf

                rbc = {}
                for s in range(S):
                    for nt in range(NT):
                        ps_b = psm.tile([P, NTILE], F32, tag="mm",
                                        name=f"ps_rb_{s}_{nt}")
                        nc.tensor.matmul(ps_b, lhsT=ones_colT,
                                         rhs=r_bfs[s][:, nt * NTILE:(nt + 1) * NTILE],
                                         start=True, stop=True)
                        rb = stream.tile([P, NTILE], F32, tag="rbc", bufs=4,
                                         name=f"rbc_{s}_{nt}")
                        nc.scalar.copy(out=rb, in_=ps_b)
                        rbc[(s, nt)] = rb

                hN = {}
                for s in range(S):
                    for cj in range(2):
                        hN[(s, cj)] = attn.tile([P, 2, NPIX], F8, tag="hn",
                                                bufs=2 * S, name=f"hn_{s}_{cj}")
                for s in range(S):
                    for cc in range(CC):
                        ps_h = [psm.tile([P, NTILE], F32, tag="mm",
                                         name=f"ps_h_{s}_{cc}_{nt}")
                                for nt in range(NT)]
                        keys = [(tcn, jp) for tcn in range(VTC) for jp in range(2)]
                        for ki, (tcn, jp) in enumerate(keys):
                            lhsT = vT8_sb[(s, tcn, jp)][:, :, cc * P:(cc + 1) * P]
                            for nt in range(NT):
                                nc.tensor.matmul(
                                    ps_h[nt], lhsT=lhsT,
                                    rhs=exps8[(s, tcn, jp)][
                                        :, :, nt * NTILE:(nt + 1) * NTILE],
                                    start=(ki == 0), stop=(ki == len(keys) - 1),
                                    perf_mode=DR)
                        for nt in range(NT):
                            nc.vector.tensor_copy(
                                out=hN[(s, cc // 2)][:, cc % 2,
                                                     nt * NTILE:(nt + 1) * NTILE],
                                in_=ps_h[nt])

                for s in range(S):
                    o_t = stream.tile([P, CC, NPIX], BF16, tag="ostage", bufs=2,
                                      name=f"o_{s}")
                    for oc in range(CC):
                        ps_p = [psm.tile([P, NTILE], F32, tag="mm",
                                         name=f"ps_p_{s}_{oc}_{nt}")
                                for nt in range(NT)]
                        for cj in range(2):
                            lhsT = wproj_sb[cj][:, :, oc * P:(oc + 1) * P]
                            for nt in range(NT):
                                nc.tensor.matmul(
                                    ps_p[nt], lhsT=lhsT,
                                    rhs=hN[(s, cj)][:, :, nt * NTILE:(nt + 1) * NTILE],
                                    start=(cj == 0), stop=(cj == 1),
                                    perf_mode=DR)
                        for nt in range(NT):
                            sl = slice(nt * NTILE, (nt + 1) * NTILE)
                            nc.vector.tensor_mul(out=o_t[:, oc, sl], in0=ps_p[nt],
                                                 in1=rbc[(s, nt)])
                        nc.scalar.dma_start(out_d[s, :, oc], o_t[:, oc])

    nc.finalize()
    return nc


BT43 = np.array([
    [4, 0, -5, 0, 1, 0],
    [0, -4, -4, 1, 1, 0],
    [0, 4, -4, -1, 1, 0],
    [0, -2, -1, 2, 1, 0],
    [0, 2, -1, -2, 1, 0],
    [0, 4, 0, -5, 0, 1]], np.float32)
G43 = np.array([
    [1 / 4, 0, 0],
    [-1 / 6, -1 / 6, -1 / 6],
    [-1 / 6, 1 / 6, -1 / 6],
    [1 / 24, 1 / 12, 1 / 6],
    [1 / 24, -1 / 12, 1 / 6],
    [0, 0, 1]], np.float32)


def prep_inputs(x, w_qkv, b_qkv):
    e4 = ml_dtypes.float8_e4m3
    xpad = np.zeros((B, C, HP, WP), np.float32)
    xpad[:, :, 1:H + 1, 1:W + 1] = x

    taps = np.stack([xpad[:, :, :, a:a + 4 * TX:4][:, :, :, :TX]
                     for a in range(6)])          # [6, B, C, HP, TX]
    V = np.tensordot(BT43, taps, axes=([1], [0]))  # [6, B, C, HP, TX]
    vw = np.ascontiguousarray(
        V.reshape(NP6, B, 2, 2, P, HP, TX)
        .transpose(0, 2, 4, 3, 1, 5, 6)).astype(e4)  # [6, 2, P, 2, B, HP, TX]

    u6 = np.tensordot(w_qkv * SU, G43, axes=([3], [1]))  # [co, ci, ky, 6]
    uw = np.ascontiguousarray(
        u6[:2 * C].reshape(QKCH, P, 2, 2, P, KY, NP6)
        .transpose(0, 4, 6, 5, 2, 3, 1)
        .reshape(QKCH, P, NP6 * KY * 2 * 2 * P)).astype(e4)
    uv = np.ascontiguousarray(
        u6[2 * C:].reshape(C, 2, 2, P, KY, NP6)
        .transpose(3, 5, 4, 1, 2, 0)
        .reshape(P, NP6 * KY * 2 * 2 * C)).astype(e4)
    bqkv = np.ascontiguousarray((b_qkv[:2 * C] * SU).reshape(QKCH, P)).astype(
        ml_dtypes.bfloat16)
    bv = np.ascontiguousarray((b_qkv[2 * C:] * SU).reshape(1, C)).astype(
        ml_dtypes.bfloat16)
    return vw, uw, uv, bqkv, bv


def kernel(x, w_qkv, b_qkv, w_proj, b_proj, gn_gamma=None, gn_beta=None):
    global LAST_EXEC_NS
    x = np.asarray(x, np.float32)
    w_qkv = np.asarray(w_qkv, np.float32)
    b_qkv = np.asarray(b_qkv, np.float32)
    w_proj = np.asarray(w_proj, np.float32)
    b_proj = np.asarray(b_proj, np.float32)

    with_bias = bool(np.any(b_qkv))
    if with_bias not in _CACHED:
        _CACHED[with_bias] = build_nc(with_bias=with_bias)
    nc = _CACHED[with_bias]

    e4 = ml_dtypes.float8_e4m3
    vw, uw, uv, bqkv, bv = prep_inputs(x, w_qkv, b_qkv)
    wproj = np.ascontiguousarray(
        (w_proj[:, :, 0, 0].T * float(1 << 21))
        .reshape(2, 2, P, C).transpose(0, 2, 1, 3)).astype(e4)

    in_maps = []
    for core in range(NCORES):
        sl = slice(core * S, (core + 1) * S)
        in_maps.append({
            "vw": np.ascontiguousarray(vw[:, :, :, :, sl]).reshape(NP6, 2, P, -1),
            "uw": uw,
            "uv": uv,
            "wproj": wproj,
            "bqkv": bqkv,
            "bv": bv,
        })

    res = run_bass_kernel_spmd(nc, in_maps, list(range(NCORES)), trace=TRACE)
    LAST_EXEC_NS = res.exec_time_ns
    h = np.stack([np.asarray(res.results[c]["out"], np.float32)
                  for c in range(NCORES)])
    h = h.reshape(B, P, CC, NPIX).transpose(0, 2, 1, 3).reshape(B, C, H, W)
    out = x + h + b_proj[None, :, None, None]
    return np.ascontiguousarray(out).astype(np.float32, copy=False)
